# revision 2
# baseline (speedup 1.0000x reference)
"""Trainium2 Bass kernel for the ControlledNODE problem.

Strategy (single trajectory, strictly sequential recurrence -> one core):
  - State h ([32] fp32) lives as a column of an SBUF history buffer.
  - Per RK4 step the critical path is 9 serialized (TensorE matmul -> ScalarE
    activation) rounds: 8 Silu + 1 Tanh. RK4 intermediate states h2/h3/h4 are
    never materialized: layer-1 preactivations are accumulated directly in
    PSUM as  p_i = alpha_i*(W1h^T h) + sum_j q_ij*(M31^T z2_j) + bias_i(t)
    with M31 = W3 @ W1h and all scalar RK4 coefficients folded into
    host-prescaled stationary weight copies.
  - h_next = Tanh(psum + bias) in one fused ACT op (clip is a no-op since
    |tanh|<=1 < 5; nan_to_num unnecessary for this bounded system).
  - Input projection au(t) = W1u^T u_t + b1 (+ per-eval consts) and the three
    linear readouts are batched per chunk of UC steps, off the critical path.
  - T/UC chunks via a tc.For_i dynamic loop, UC steps unrolled per body.
"""

import numpy as np

HDIM, U_DIM, HID = 32, 8, 128
DYN_GAIN = 0.02
STATE_DAMP = 0.1
DT = float(np.float32(5.0 / 60.0))


def rk4_coeffs():
    """Coefficients of (h2,h3,h4,hpre) over basis [h, E1, E2, E3, E4, b3],
    where E_i = W3^T z2_i and k_i = c*E_i + c*b3 - d*h_i."""
    c, d, dt = DYN_GAIN, STATE_DAMP, DT

    def v(*a):
        return np.array(a, dtype=np.float64)

    h1 = v(1, 0, 0, 0, 0, 0)
    b3v = v(0, 0, 0, 0, 0, 1)
    E = [v(0, 1, 0, 0, 0, 0), v(0, 0, 1, 0, 0, 0),
         v(0, 0, 0, 1, 0, 0), v(0, 0, 0, 0, 1, 0)]

    def k(i, hi):
        return c * E[i] + c * b3v - d * hi

    k1 = k(0, h1)
    h2 = h1 + (dt / 2) * k1
    k2 = k(1, h2)
    h3 = h1 + (dt / 2) * k2
    k3 = k(2, h3)
    h4 = h1 + dt * k3
    k4 = k(3, h4)
    hpre = h1 + (dt / 6) * (k1 + 2 * k2 + 2 * k3 + k4)
    return [h1, h2, h3, h4], hpre


def host_prep(U, h0, W1, b1, W2, b2, W3, b3, Wd, bd, Wt, bt, Wc, bc):
    """Fold RK4 coefficients into prescaled weight copies; returns the
    input map for the device program plus the scalar A coefficient."""
    f32 = np.float32
    asf = lambda x: np.ascontiguousarray(np.asarray(x), dtype=np.float32)
    U, W1, W2, W3 = asf(U), asf(W1), asf(W2), asf(W3)
    b1, b2, b3 = asf(b1), asf(b2), asf(b3)
    W1h, W1u = W1[:HDIM], W1[HDIM:]
    M31 = (W3 @ W1h).astype(f32)              # [128,128]
    v1b3 = (W1h.T @ b3).astype(f32)           # [128]
    hcoef, hpre_c = rk4_coeffs()

    inmap = {"U": U, "W1h": np.ascontiguousarray(W1h)}
    for i in range(4):
        for j in range(i):
            inmap[f"M31s{i}{j}"] = np.ascontiguousarray(f32(hcoef[i][1 + j]) * M31)
    inmap["W2"] = W2
    for j in range(4):
        inmap[f"W3s{j}"] = np.ascontiguousarray(f32(hpre_c[1 + j]) * W3)
    inmap["W1u"] = np.ascontiguousarray(W1u)
    Wdtc = np.stack([asf(Wd)[:, 0], asf(Wt)[:, 0], asf(Wc)[:, 0]], axis=1)
    inmap["Wdtc"] = np.ascontiguousarray(Wdtc)          # [32,3]
    for i in range(4):  # b1 + r_i * (W1h^T b3), column-shaped
        inmap[f"cb1_{i}"] = np.ascontiguousarray(
            (b1 + f32(hcoef[i][5]) * v1b3).reshape(HID, 1))
    inmap["b2v"] = np.ascontiguousarray(b2.reshape(HID, 1))
    inmap["Rb3"] = np.ascontiguousarray((f32(hpre_c[5]) * b3).reshape(HDIM, 1))
    bdtc = np.array([asf(bd)[0], asf(bt)[0], asf(bc)[0]], f32)
    inmap["bdtc"] = np.ascontiguousarray(bdtc.reshape(3, 1))
    inmap["h0"] = asf(h0)                                # [1,32]
    A = float(f32(hpre_c[0]))
    return inmap, A


def build_program(T, UC):
    """Build the Bass program; returns (nc, out_names)."""
    import concourse.bass as bass
    import concourse.bacc as bacc
    import concourse.mybir as mybir
    from concourse.tile import TileContext

    f32 = mybir.dt.float32
    AF = mybir.ActivationFunctionType
    ALU = mybir.AluOpType
    NCH = T // UC
    assert T % UC == 0

    _, hpre_c = rk4_coeffs()
    A = float(np.float32(hpre_c[0]))

    nc = bacc.Bacc("TRN2", target_bir_lowering=False, debug=False,
                   enable_asserts=False, num_devices=1)

    # --- DRAM tensors -----------------------------------------------------
    d_in = {}

    def din(name, shape):
        d_in[name] = nc.dram_tensor(name, list(shape), f32, kind="ExternalInput").ap()

    din("U", (T, U_DIM))
    for i in range(4):
        din(f"W1hs{i}", (HDIM, HID))
    for i in range(4):
        for j in range(i):
            din(f"M31s{i}{j}", (HID, HID))
    din("W2", (HID, HID))
    for j in range(4):
        din(f"W3s{j}", (HID, HDIM))
    din("W1u", (U_DIM, HID))
    din("Wdtc", (HDIM, 3))
    for i in range(4):
        din(f"cb1_{i}", (HID, 1))
    din("b2v", (HID, 1))
    din("Rb3", (HDIM, 1))
    din("bdtc", (3, 1))
    din("h0", (1, HDIM))

    dtc_out = nc.dram_tensor("dtc", [3, T], f32, kind="ExternalOutput").ap()
    hlast_out = nc.dram_tensor("hlast", [1, HDIM], f32, kind="ExternalOutput").ap()

    with TileContext(nc) as tc:
        with (
            tc.tile_pool(name="stat", bufs=1) as stat,
            tc.tile_pool(name="chunk", bufs=1) as chunkp,
            tc.tile_pool(name="zs", bufs=2) as zpool,
            tc.tile_pool(name="pp", bufs=2, space="PSUM") as psum_p,
            tc.tile_pool(name="py", bufs=2, space="PSUM") as psum_y,
            tc.tile_pool(name="ph", bufs=2, space="PSUM") as psum_h,
            tc.tile_pool(name="pau", bufs=1, space="PSUM") as psum_au,
        ):
            # --- persistent SBUF: stationaries, const vectors, history ----
            def load(name, shape):
                t = stat.tile(list(shape), f32, tag=name)
                nc.sync.dma_start(t[:, :], d_in[name][:, :])
                return t

            W1hs = [load(f"W1hs{i}", (HDIM, HID)) for i in range(4)]
            M31s = {(i, j): load(f"M31s{i}{j}", (HID, HID))
                    for i in range(4) for j in range(i)}
            W2s = load("W2", (HID, HID))
            W3s = [load(f"W3s{j}", (HID, HDIM)) for j in range(4)]
            W1us = load("W1u", (U_DIM, HID))
            Wdtcs = load("Wdtc", (HDIM, 3))
            cb1 = [load(f"cb1_{i}", (HID, 1)) for i in range(4)]
            b2v = load("b2v", (HID, 1))
            Rb3 = load("Rb3", (HDIM, 1))
            bdtc = load("bdtc", (3, 1))

            H = stat.tile([HDIM, UC + 1], f32, tag="hist")
            # h0 [1,32] -> H[:,0] (transpose via access pattern)
            nc.sync.dma_start(H[:, 0:1], d_in["h0"].rearrange("a b -> b a"))

            # Warm the ACT table set (silu_and_others: Silu+Tanh+Identity)
            # outside the loop so no PSEUDO_LOAD lands inside the body.
            scratch = stat.tile([HDIM, 1], f32, tag="scratch")
            nc.vector.memset(scratch[:, :], 0.0)
            nc.scalar.activation(scratch[:, :], scratch[:, :], AF.Silu)
            nc.scalar.activation(scratch[:, :], scratch[:, :], AF.Tanh)

            UT = d_in["U"].rearrange("t u -> u t")      # [8, T] view

            with tc.For_i(0, NCH, 1,
                          hint_engines=(mybir.EngineType.PE,
                                        mybir.EngineType.Activation)) as ci:
                # ---- chunk phase: au projection + 4 bias streams ---------
                Usb = chunkp.tile([U_DIM, UC], f32, tag="Usb")
                nc.sync.dma_start(Usb[:, :], UT[:, bass.ts(ci, UC)])
                au_ps = psum_au.tile([HID, UC], f32, tag="aups")
                nc.tensor.matmul(au_ps[:, :], W1us[:, :], Usb[:, :],
                                 start=True, stop=True)
                Bias = []
                for i in range(4):
                    bi = chunkp.tile([HID, UC], f32, tag=f"bias{i}")
                    nc.vector.tensor_scalar(bi[:, :], au_ps[:, :],
                                            cb1[i][:, :], None, ALU.add)
                    Bias.append(bi)

                # ---- UC recurrence steps ---------------------------------
                for s in range(UC):
                    h_ap = H[:, s:s + 1]
                    bias_h = zpool.tile([HDIM, 1], f32, tag="biash")
                    nc.vector.tensor_scalar(bias_h[:, :], h_ap, A, Rb3[:, :],
                                            ALU.mult, ALU.add)
                    z2s = []
                    hp_ps = None
                    for i in range(4):
                        p_ps = psum_p.tile([HID, 1], f32, tag="p")
                        nc.tensor.matmul(p_ps[:, :], W1hs[i][:, :], h_ap,
                                         start=True, stop=(i == 0))
                        for j in range(i):
                            nc.tensor.matmul(p_ps[:, :], M31s[(i, j)][:, :],
                                             z2s[j][:, :], start=False,
                                             stop=(j == i - 1))
                        z1 = zpool.tile([HID, 1], f32, tag="z1")
                        nc.scalar.activation(z1[:, :], p_ps[:, :], AF.Silu,
                                             bias=Bias[i][:, s:s + 1], scale=1.0)
                        y_ps = psum_y.tile([HID, 1], f32, tag="y")
                        nc.tensor.matmul(y_ps[:, :], W2s[:, :], z1[:, :],
                                         start=True, stop=True)
                        z2 = zpool.tile([HID, 1], f32, tag=f"z2_{i}")
                        nc.scalar.activation(z2[:, :], y_ps[:, :], AF.Silu,
                                             bias=b2v[:, :], scale=1.0)
                        z2s.append(z2)
                        if i == 0:
                            hp_ps = psum_h.tile([HDIM, 1], f32, tag="hp")
                        nc.tensor.matmul(hp_ps[:, :], W3s[i][:, :], z2[:, :],
                                         start=(i == 0), stop=(i == 3))
                    nc.scalar.activation(H[:, s + 1:s + 2], hp_ps[:, :],
                                         AF.Tanh, bias=bias_h[:, :], scale=1.0)

                # ---- readouts + output DMA + history carry ---------------
                ro_ps = psum_au.tile([3, UC], f32, tag="aups")
                nc.tensor.matmul(ro_ps[:, :], Wdtcs[:, :], H[:, 0:UC],
                                 start=True, stop=True)
                dtc_sb = chunkp.tile([3, UC], f32, tag="dtcsb")
                nc.scalar.activation(dtc_sb[:, :], ro_ps[:, :], AF.Identity,
                                     bias=bdtc[:, :], scale=1.0)
                nc.sync.dma_start(dtc_out[:, bass.ts(ci, UC)], dtc_sb[:, :])
                nc.vector.tensor_copy(H[:, 0:1], H[:, UC:UC + 1])

            nc.sync.dma_start(hlast_out.rearrange("a b -> b a"), H[:, 0:1])

    nc.compile()
    return nc


_UC = 128


def kernel(**inputs):
    T = inputs["U"].shape[0]
    inmap, _A = host_prep(**inputs)
    nc = build_program(T, _UC)
    from concourse import bass_utils
    res = bass_utils.run_bass_kernel_spmd(nc, [inmap], core_ids=[0]).results[0]
    dtc = res["dtc"]
    return (np.ascontiguousarray(dtc[0]), np.ascontiguousarray(dtc[1]),
            np.ascontiguousarray(dtc[2]), res["hlast"])


if __name__ == "__main__":
    import sys
    T = int(sys.argv[1]) if len(sys.argv) > 1 else 512
    rng = np.random.RandomState(0)
    U = rng.randn(T, U_DIM).astype(np.float32)
    demo = dict(
        U=U, h0=np.zeros((1, HDIM), np.float32),
        W1=0.1 * rng.randn(HDIM + U_DIM, HID).astype(np.float32),
        b1=0.02 * rng.randn(HID).astype(np.float32),
        W2=0.1 * rng.randn(HID, HID).astype(np.float32),
        b2=0.02 * rng.randn(HID).astype(np.float32),
        W3=0.1 * rng.randn(HID, HDIM).astype(np.float32),
        b3=0.02 * rng.randn(HDIM).astype(np.float32),
        Wd=0.1 * rng.randn(HDIM, 1).astype(np.float32),
        bd=np.zeros(1, np.float32),
        Wt=0.1 * rng.randn(HDIM, 1).astype(np.float32),
        bt=np.zeros(1, np.float32),
        Wc=0.1 * rng.randn(HDIM, 1).astype(np.float32),
        bc=np.zeros(1, np.float32),
    )
    from proto import kernel_proto
    exp = kernel_proto(**demo)
    act = kernel(**demo)
    for n, e, a in zip(["ds", "ts", "cs", "h_last"], exp, act):
        err = np.abs(a - e).max()
        print(f"{n}: absmax_vs_proto={err:.3e} scale={np.abs(e).max():.3e}")


# revision 8
# speedup vs baseline: 1.1135x; 1.1135x over previous
"""Trainium2 Bass kernel for the ControlledNODE problem.

Strategy (single trajectory, strictly sequential recurrence -> one core):
  - State h ([32] fp32) lives as a column of an SBUF history buffer.
  - Per RK4 step the critical path is 9 serialized (TensorE matmul -> ScalarE
    activation) rounds: 8 Silu + 1 Tanh. RK4 intermediate states h2/h3/h4 are
    never materialized: layer-1 preactivations are accumulated directly in
    PSUM as  p_i = alpha_i*(W1h^T h) + sum_j q_ij*(M31^T z2_j) + bias_i(t)
    with M31 = W3 @ W1h and all scalar RK4 coefficients folded into
    host-prescaled stationary weight copies.
  - h_next = Tanh(psum + bias) in one fused ACT op (clip is a no-op since
    |tanh|<=1 < 5; nan_to_num unnecessary for this bounded system).
  - Input projection au(t) = W1u^T u_t + b1 (+ per-eval consts) and the three
    linear readouts are batched per chunk of UC steps, off the critical path.
  - T/UC chunks via a tc.For_i dynamic loop, UC steps unrolled per body.
"""

import numpy as np

HDIM, U_DIM, HID = 32, 8, 128
DYN_GAIN = 0.02
STATE_DAMP = 0.1
DT = float(np.float32(5.0 / 60.0))


def rk4_coeffs():
    """Coefficients of (h2,h3,h4,hpre) over basis [h, E1, E2, E3, E4, b3],
    where E_i = W3^T z2_i and k_i = c*E_i + c*b3 - d*h_i."""
    c, d, dt = DYN_GAIN, STATE_DAMP, DT

    def v(*a):
        return np.array(a, dtype=np.float64)

    h1 = v(1, 0, 0, 0, 0, 0)
    b3v = v(0, 0, 0, 0, 0, 1)
    E = [v(0, 1, 0, 0, 0, 0), v(0, 0, 1, 0, 0, 0),
         v(0, 0, 0, 1, 0, 0), v(0, 0, 0, 0, 1, 0)]

    def k(i, hi):
        return c * E[i] + c * b3v - d * hi

    k1 = k(0, h1)
    h2 = h1 + (dt / 2) * k1
    k2 = k(1, h2)
    h3 = h1 + (dt / 2) * k2
    k3 = k(2, h3)
    h4 = h1 + dt * k3
    k4 = k(3, h4)
    hpre = h1 + (dt / 6) * (k1 + 2 * k2 + 2 * k3 + k4)
    return [h1, h2, h3, h4], hpre


def host_prep(U, h0, W1, b1, W2, b2, W3, b3, Wd, bd, Wt, bt, Wc, bc):
    """Fold RK4 coefficients into prescaled weight copies; returns the
    input map for the device program plus the scalar A coefficient."""
    f32 = np.float32
    asf = lambda x: np.ascontiguousarray(np.asarray(x), dtype=np.float32)
    U, W1, W2, W3 = asf(U), asf(W1), asf(W2), asf(W3)
    b1, b2, b3 = asf(b1), asf(b2), asf(b3)
    W1h, W1u = W1[:HDIM], W1[HDIM:]
    M31 = (W3 @ W1h).astype(f32)              # [128,128]
    v1b3 = (W1h.T @ b3).astype(f32)           # [128]
    hcoef, hpre_c = rk4_coeffs()

    inmap = {"U": U, "W1h": np.ascontiguousarray(W1h)}
    for i in range(4):
        for j in range(i):
            inmap[f"M31s{i}{j}"] = np.ascontiguousarray(f32(hcoef[i][1 + j]) * M31)
    inmap["W2"] = W2
    for j in range(4):
        inmap[f"W3s{j}"] = np.ascontiguousarray(f32(hpre_c[1 + j]) * W3)
    inmap["W1u"] = np.ascontiguousarray(W1u)
    Wdtc = np.stack([asf(Wd)[:, 0], asf(Wt)[:, 0], asf(Wc)[:, 0]], axis=1)
    inmap["Wdtc"] = np.ascontiguousarray(Wdtc)          # [32,3]
    for i in range(4):  # b1 + r_i * (W1h^T b3), column-shaped
        inmap[f"cb1_{i}"] = np.ascontiguousarray(
            (b1 + f32(hcoef[i][5]) * v1b3).reshape(HID, 1))
    inmap["b2v"] = np.ascontiguousarray(b2.reshape(HID, 1))
    inmap["Rb3"] = np.ascontiguousarray((f32(hpre_c[5]) * b3).reshape(HDIM, 1))
    bdtc = np.array([asf(bd)[0], asf(bt)[0], asf(bc)[0]], f32)
    inmap["bdtc"] = np.ascontiguousarray(bdtc.reshape(3, 1))
    inmap["h0"] = asf(h0)                                # [1,32]
    A = float(f32(hpre_c[0]))
    return inmap, A


def build_program(T, UC, static_loop=False):
    """Build the Bass program; returns (nc, out_names)."""
    import concourse.bass as bass
    import concourse.bacc as bacc
    import concourse.mybir as mybir
    from concourse.tile import TileContext

    f32 = mybir.dt.float32
    AF = mybir.ActivationFunctionType
    ALU = mybir.AluOpType
    NCH = T // UC
    assert T % UC == 0

    hcoef, hpre_c = rk4_coeffs()
    A = float(np.float32(hpre_c[0]))
    alphas = [float(np.float32(hcoef[i][0])) for i in range(4)]

    nc = bacc.Bacc("TRN2", target_bir_lowering=False, debug=False,
                   enable_asserts=False, num_devices=1)

    # --- DRAM tensors -----------------------------------------------------
    d_in = {}

    def din(name, shape):
        d_in[name] = nc.dram_tensor(name, list(shape), f32, kind="ExternalInput").ap()

    din("U", (T, U_DIM))
    din("W1h", (HDIM, HID))
    for i in range(4):
        for j in range(i):
            din(f"M31s{i}{j}", (HID, HID))
    din("W2", (HID, HID))
    for j in range(4):
        din(f"W3s{j}", (HID, HDIM))
    din("W1u", (U_DIM, HID))
    din("Wdtc", (HDIM, 3))
    for i in range(4):
        din(f"cb1_{i}", (HID, 1))
    din("b2v", (HID, 1))
    din("Rb3", (HDIM, 1))
    din("bdtc", (3, 1))
    din("h0", (1, HDIM))

    dtc_out = nc.dram_tensor("dtc", [3, T], f32, kind="ExternalOutput").ap()
    hlast_out = nc.dram_tensor("hlast", [1, HDIM], f32, kind="ExternalOutput").ap()

    with TileContext(nc) as tc:
        with (
            tc.tile_pool(name="stat", bufs=1) as stat,
            tc.tile_pool(name="chunk", bufs=1) as chunkp,
            tc.tile_pool(name="zs", bufs=2) as zpool,
            tc.tile_pool(name="pp", bufs=3, space="PSUM") as psum_p,
            tc.tile_pool(name="py", bufs=2, space="PSUM") as psum_y,
            tc.tile_pool(name="ph", bufs=2, space="PSUM") as psum_h,
            tc.tile_pool(name="pau", bufs=1, space="PSUM") as psum_au,
        ):
            # --- persistent SBUF: stationaries, const vectors, history ----
            def load(name, shape):
                t = stat.tile(list(shape), f32, tag=name)
                nc.sync.dma_start(t[:, :], d_in[name][:, :])
                return t

            W1hs = load("W1h", (HDIM, HID))
            M31s = {(i, j): load(f"M31s{i}{j}", (HID, HID))
                    for i in range(4) for j in range(i)}
            W2s = load("W2", (HID, HID))
            W3s = [load(f"W3s{j}", (HID, HDIM)) for j in range(4)]
            W1us = load("W1u", (U_DIM, HID))
            Wdtcs = load("Wdtc", (HDIM, 3))
            cb1 = [load(f"cb1_{i}", (HID, 1)) for i in range(4)]
            b2v = load("b2v", (HID, 1))
            Rb3 = load("Rb3", (HDIM, 1))
            bdtc = load("bdtc", (3, 1))

            H = stat.tile([HDIM, UC + 1], f32, tag="hist")
            # h0 [1,32] -> H[:,0] (transpose via access pattern)
            nc.sync.dma_start(H[:, 0:1], d_in["h0"].rearrange("a b -> b a"))

            # Warm the ACT table set (silu_and_others: Silu+Tanh+Identity)
            # outside the loop so no PSEUDO_LOAD lands inside the body.
            scratch = stat.tile([HDIM, 1], f32, tag="scratch")
            nc.vector.memset(scratch[:, :], 0.0)
            nc.scalar.activation(scratch[:, :], scratch[:, :], AF.Silu)
            nc.scalar.activation(scratch[:, :], scratch[:, :], AF.Tanh)

            UT = d_in["U"].rearrange("t u -> u t")      # [8, T] view

            from contextlib import contextmanager

            @contextmanager
            def chunk_iter():
                if static_loop:
                    def body_runner(fn):
                        for civ in range(NCH):
                            fn(civ)
                    yield body_runner
                else:
                    def body_runner(fn):
                        with tc.For_i(0, NCH, 1,
                                      hint_engines=(mybir.EngineType.PE,
                                                    mybir.EngineType.Activation)) as ci:
                            fn(ci)
                    yield body_runner

            def chunk_body(ci):
                # ---- chunk phase: au projection + 4 bias streams ---------
                Usb = chunkp.tile([U_DIM, UC], f32, tag="Usb")
                nc.sync.dma_start(Usb[:, :], UT[:, bass.ts(ci, UC)])
                au_ps = psum_au.tile([HID, UC], f32, tag="aups")
                nc.tensor.matmul(au_ps[:, :], W1us[:, :], Usb[:, :],
                                 start=True, stop=True)
                Bias = []
                for i in range(4):
                    bi = chunkp.tile([HID, UC], f32, tag=f"bias{i}")
                    nc.vector.tensor_scalar(bi[:, :], au_ps[:, :],
                                            cb1[i][:, :], None, ALU.add)
                    Bias.append(bi)

                # ---- UC recurrence steps ---------------------------------
                # Critical path per step: 9 (PE matmul -> ACT) rounds.
                # Emission order keeps each round's critical matmul FIRST
                # after its dependency; off-path matmuls/DVE ops trail.
                for s in range(UC):
                    h_ap = H[:, s:s + 1]
                    bias_h = zpool.tile([HDIM, 1], f32, tag="biash")
                    nc.vector.tensor_scalar(bias_h[:, :], h_ap, A, Rb3[:, :],
                                            ALU.mult, ALU.add)
                    # R1: p1 = G = W1h^T h
                    p1 = psum_p.tile([HID, 1], f32, tag="p")
                    nc.tensor.matmul(p1[:, :], W1hs[:, :], h_ap,
                                     start=True, stop=True)
                    z1_1 = zpool.tile([HID, 1], f32, tag="z1")
                    nc.scalar.activation(z1_1[:, :], p1[:, :], AF.Silu,
                                         bias=Bias[0][:, s:s + 1], scale=1.0)
                    # G-based silu biases for evals 2-4 (off critical path)
                    G = zpool.tile([HID, 1], f32, tag="G")
                    nc.vector.tensor_copy(G[:, :], p1[:, :])
                    bias_e = [None] * 4
                    for i in range(1, 4):
                        be = zpool.tile([HID, 1], f32, tag=f"be{i}")
                        nc.vector.tensor_scalar(be[:, :], G[:, :], alphas[i],
                                                Bias[i][:, s:s + 1],
                                                ALU.mult, ALU.add)
                        bias_e[i] = be
                    # R2: y1
                    y1 = psum_y.tile([HID, 1], f32, tag="y")
                    nc.tensor.matmul(y1[:, :], W2s[:, :], z1_1[:, :],
                                     start=True, stop=True)
                    z2_1 = zpool.tile([HID, 1], f32, tag="z2_0")
                    nc.scalar.activation(z2_1[:, :], y1[:, :], AF.Silu,
                                         bias=b2v[:, :], scale=1.0)
                    # R3: p2 (critical), then off-path z2_1 consumers
                    p2 = psum_p.tile([HID, 1], f32, tag="p")
                    nc.tensor.matmul(p2[:, :], M31s[(1, 0)][:, :], z2_1[:, :],
                                     start=True, stop=True)
                    z1_2 = zpool.tile([HID, 1], f32, tag="z1")
                    nc.scalar.activation(z1_2[:, :], p2[:, :], AF.Silu,
                                         bias=bias_e[1][:, :], scale=1.0)
                    p3 = psum_p.tile([HID, 1], f32, tag="p")
                    nc.tensor.matmul(p3[:, :], M31s[(2, 0)][:, :], z2_1[:, :],
                                     start=True, stop=False)
                    hp = psum_h.tile([HDIM, 1], f32, tag="hp")
                    nc.tensor.matmul(hp[:, :], W3s[0][:, :], z2_1[:, :],
                                     start=True, stop=False)
                    # R4: y2
                    y2 = psum_y.tile([HID, 1], f32, tag="y")
                    nc.tensor.matmul(y2[:, :], W2s[:, :], z1_2[:, :],
                                     start=True, stop=True)
                    z2_2 = zpool.tile([HID, 1], f32, tag="z2_1")
                    nc.scalar.activation(z2_2[:, :], y2[:, :], AF.Silu,
                                         bias=b2v[:, :], scale=1.0)
                    # R5: p3 finish (critical), p4 starts, hp, off-path
                    nc.tensor.matmul(p3[:, :], M31s[(2, 1)][:, :], z2_2[:, :],
                                     start=False, stop=True)
                    z1_3 = zpool.tile([HID, 1], f32, tag="z1")
                    nc.scalar.activation(z1_3[:, :], p3[:, :], AF.Silu,
                                         bias=bias_e[2][:, :], scale=1.0)
                    p4 = psum_p.tile([HID, 1], f32, tag="p")
                    nc.tensor.matmul(p4[:, :], M31s[(3, 0)][:, :], z2_1[:, :],
                                     start=True, stop=False)
                    nc.tensor.matmul(p4[:, :], M31s[(3, 1)][:, :], z2_2[:, :],
                                     start=False, stop=False)
                    nc.tensor.matmul(hp[:, :], W3s[1][:, :], z2_2[:, :],
                                     start=False, stop=False)
                    # R6: y3
                    y3 = psum_y.tile([HID, 1], f32, tag="y")
                    nc.tensor.matmul(y3[:, :], W2s[:, :], z1_3[:, :],
                                     start=True, stop=True)
                    z2_3 = zpool.tile([HID, 1], f32, tag="z2_2")
                    nc.scalar.activation(z2_3[:, :], y3[:, :], AF.Silu,
                                         bias=b2v[:, :], scale=1.0)
                    # R7: p4 finish (critical), hp
                    nc.tensor.matmul(p4[:, :], M31s[(3, 2)][:, :], z2_3[:, :],
                                     start=False, stop=True)
                    z1_4 = zpool.tile([HID, 1], f32, tag="z1")
                    nc.scalar.activation(z1_4[:, :], p4[:, :], AF.Silu,
                                         bias=bias_e[3][:, :], scale=1.0)
                    nc.tensor.matmul(hp[:, :], W3s[2][:, :], z2_3[:, :],
                                     start=False, stop=False)
                    # R8: y4
                    y4 = psum_y.tile([HID, 1], f32, tag="y")
                    nc.tensor.matmul(y4[:, :], W2s[:, :], z1_4[:, :],
                                     start=True, stop=True)
                    z2_4 = zpool.tile([HID, 1], f32, tag="z2_3")
                    nc.scalar.activation(z2_4[:, :], y4[:, :], AF.Silu,
                                         bias=b2v[:, :], scale=1.0)
                    # R9: hp finish (critical) -> fused Tanh state update
                    nc.tensor.matmul(hp[:, :], W3s[3][:, :], z2_4[:, :],
                                     start=False, stop=True)
                    nc.scalar.activation(H[:, s + 1:s + 2], hp[:, :],
                                         AF.Tanh, bias=bias_h[:, :], scale=1.0)

                # ---- readouts + output DMA + history carry ---------------
                ro_ps = psum_au.tile([3, UC], f32, tag="aups")
                nc.tensor.matmul(ro_ps[:, :], Wdtcs[:, :], H[:, 0:UC],
                                 start=True, stop=True)
                dtc_sb = chunkp.tile([3, UC], f32, tag="dtcsb")
                nc.scalar.activation(dtc_sb[:, :], ro_ps[:, :], AF.Identity,
                                     bias=bdtc[:, :], scale=1.0)
                nc.sync.dma_start(dtc_out[:, bass.ts(ci, UC)], dtc_sb[:, :])
                nc.vector.tensor_copy(H[:, 0:1], H[:, UC:UC + 1])

            with chunk_iter() as runner:
                runner(chunk_body)

            nc.sync.dma_start(hlast_out.rearrange("a b -> b a"), H[:, 0:1])

    nc.compile()
    return nc


_UC = 128


def kernel(**inputs):
    T = inputs["U"].shape[0]
    inmap, _A = host_prep(**inputs)
    nc = build_program(T, _UC)
    from concourse import bass_utils
    res = bass_utils.run_bass_kernel_spmd(nc, [inmap], core_ids=[0]).results[0]
    dtc = res["dtc"]
    return (np.ascontiguousarray(dtc[0]), np.ascontiguousarray(dtc[1]),
            np.ascontiguousarray(dtc[2]), res["hlast"])


if __name__ == "__main__":
    import sys
    T = int(sys.argv[1]) if len(sys.argv) > 1 else 512
    rng = np.random.RandomState(0)
    U = rng.randn(T, U_DIM).astype(np.float32)
    demo = dict(
        U=U, h0=np.zeros((1, HDIM), np.float32),
        W1=0.1 * rng.randn(HDIM + U_DIM, HID).astype(np.float32),
        b1=0.02 * rng.randn(HID).astype(np.float32),
        W2=0.1 * rng.randn(HID, HID).astype(np.float32),
        b2=0.02 * rng.randn(HID).astype(np.float32),
        W3=0.1 * rng.randn(HID, HDIM).astype(np.float32),
        b3=0.02 * rng.randn(HDIM).astype(np.float32),
        Wd=0.1 * rng.randn(HDIM, 1).astype(np.float32),
        bd=np.zeros(1, np.float32),
        Wt=0.1 * rng.randn(HDIM, 1).astype(np.float32),
        bt=np.zeros(1, np.float32),
        Wc=0.1 * rng.randn(HDIM, 1).astype(np.float32),
        bc=np.zeros(1, np.float32),
    )
    from proto import kernel_proto
    exp = kernel_proto(**demo)
    act = kernel(**demo)
    for n, e, a in zip(["ds", "ts", "cs", "h_last"], exp, act):
        err = np.abs(a - e).max()
        print(f"{n}: absmax_vs_proto={err:.3e} scale={np.abs(e).max():.3e}")
